# revision 12
# baseline (speedup 1.0000x reference)
"""Trainium2 Bass kernel for MultiHeadAttention with T5-style relative position
bias and causal mask.  B=1, S=4096, D=1024, H=16, dk=64.

Sharding: tensor-parallel over heads — 2 heads per NeuronCore (8 cores).
Each core:
  - projects full [S, D] inputs against its [D, 128] weight slices (Q/K/V)
  - runs causal attention for its 2 heads in transposed layout E^T[k, q]
    (so the probability matrix lands directly in the layout the PV matmul
    needs — no on-chip transposes of P)
  - RPE bias + causal mask are a single multiplicative Toeplitz table
    G[p, j] = exp(bias(d)) (0 where masked), window-sliced per tile
  - row sums come from an appended ones-column on V (row 64 of the PV psum)
  - output projection emits a partial out^T[do, q]; host sums the 8 partials
"""

import math

import numpy as np

import concourse.bacc as bacc
import concourse.mybir as mybir
import concourse.tile as tile
from concourse.bass_utils import run_bass_kernel_spmd
from concourse.masks import make_identity

S = 4096          # sequence length
D = 1024          # model dim
H = 16            # heads
DK = 64           # head dim
P = 128           # partitions
NCORES = 8
HPC = 2           # heads per core
QC = 512          # q-chunk width
NQ = S // QC      # 8 q-chunks
NKT = S // P      # 32 k-tiles
NUM_BUCKETS = 32
MAX_DISTANCE = 4096
SHIFT = 5.0       # exp(E - SHIFT) keeps P comfortably inside fp16 range
GW = S + 384      # Toeplitz table width: window offset 384+512*qc-128*kj in [0, GW-512]
VW = 65           # V columns per tile incl. ones column
F32 = mybir.dt.float32
F32R = mybir.dt.float32r
F16 = mybir.dt.float16


def _build_program():
    nc = bacc.Bacc("TRN2", target_bir_lowering=False, debug=False,
                   num_devices=NCORES)

    xq = nc.dram_tensor("xq", [D, S], F16, kind="ExternalInput")
    xk = nc.dram_tensor("xk", [D, S], F16, kind="ExternalInput")
    xv = nc.dram_tensor("xv", [D, S], F16, kind="ExternalInput")
    wq = nc.dram_tensor("wq", [P, D], F16, kind="ExternalInput")
    wk = nc.dram_tensor("wk", [P, D], F16, kind="ExternalInput")
    wv = nc.dram_tensor("wv", [P, D], F16, kind="ExternalInput")
    wo = nc.dram_tensor("wo", [P, D], F16, kind="ExternalInput")
    bq = nc.dram_tensor("bq", [P, 1], F32, kind="ExternalInput")
    bk = nc.dram_tensor("bk", [P, 1], F32, kind="ExternalInput")
    bv = nc.dram_tensor("bv", [P, 1], F32, kind="ExternalInput")
    gt = nc.dram_tensor("gt", [P, HPC * GW], F16, kind="ExternalInput")
    out = nc.dram_tensor("out", [D, S], F16, kind="ExternalOutput")

    Exp = mybir.ActivationFunctionType.Exp
    Copy = mybir.ActivationFunctionType.Copy
    Ident = mybir.ActivationFunctionType.Identity

    with tile.TileContext(nc) as tc:
        with (
            tc.tile_pool(name="const", bufs=1) as const,
            tc.tile_pool(name="xin", bufs=3) as xin,
            tc.tile_pool(name="work", bufs=3) as work,
            tc.tile_pool(name="norm", bufs=4) as norm,
            tc.tile_pool(name="outp", bufs=2) as outp,
            tc.tile_pool(name="drbnc", bufs=4, space="DRAM") as drbnc,
        ):
            # ---- persistent SBUF tensors -------------------------------
            w_sb = {}
            for name, dram in (("wq", wq), ("wk", wk), ("wv", wv), ("wo", wo)):
                t = const.tile([P, D], F16, tag=name)
                nc.sync.dma_start(t[:], dram.ap())
                w_sb[name] = t
            b_sb = {}
            for name, dram in (("bq", bq), ("bk", bk), ("bv", bv)):
                t = const.tile([P, 1], F32, tag=name)
                nc.sync.dma_start(t[:], dram.ap())
                b_sb[name] = t
            g_sb = const.tile([P, HPC * GW], F16, tag="gt")
            nc.sync.dma_start(g_sb[:], gt.ap())
            shift_sb = const.tile([P, 1], F32, tag="shift")
            nc.vector.memset(shift_sb[:], -SHIFT)

            qt_sb = const.tile([P, S], F32R, tag="qt")   # Q^T (scaled, +bias)
            kt_sb = const.tile([P, S], F32R, tag="kt")   # K^T (+bias)
            vt_sb = const.tile([P, S], F16, tag="vt")    # V^T (+bias)
            vaug_sb = const.tile([P, HPC * VW * NKT], F16, tag="vaug")
            ctx_sb = const.tile([P, S], F16, tag="ctx")  # normalized ctx^T

            # ---- phase A: projections ---------------------------------
            with tc.tile_pool(name="pproj", bufs=4, space="PSUM") as pproj:
                for name, xdram, bias, scale, dest in (
                    ("wq", xq, "bq", 0.125, qt_sb),
                    ("wk", xk, "bk", 1.0, kt_sb),
                    ("wv", xv, "bv", 1.0, vt_sb),
                ):
                    xv_view = xdram.ap().rearrange(
                        "(c p) (s f) -> p c s f", p=P, f=QC)
                    for qc in range(NQ):
                        xt = xin.tile([P, D // P, QC], F16, tag="xchunk")
                        nc.sync.dma_start(xt[:], xv_view[:, :, qc, :])
                        ps = pproj.tile([P, QC], F32, tag="pproj")
                        for dk_i in range(D // P):
                            nc.tensor.matmul(
                                ps[:],
                                w_sb[name][:, dk_i * P:(dk_i + 1) * P],
                                xt[:, dk_i, :],
                                start=(dk_i == 0),
                                stop=(dk_i == D // P - 1),
                            )
                        nc.scalar.activation(
                            dest[:, qc * QC:(qc + 1) * QC], ps[:], Ident,
                            bias=b_sb[bias][:], scale=scale)

            # ---- V^T -> V tiles with ones column (PE transpose) -------
            ident = const.tile([P, P], F16, tag="ident")
            make_identity(nc, ident[:])
            with tc.tile_pool(name="ptr", bufs=4, space="PSUM") as ptr:
                for kj in range(NKT):
                    tr_ps = ptr.tile([P, P], F16, tag="trps")
                    nc.tensor.transpose(
                        tr_ps[:], vt_sb[:, kj * P:(kj + 1) * P], ident[:])
                    for h in range(HPC):
                        dst = vaug_sb[:, h * VW * NKT + kj * VW:
                                      h * VW * NKT + kj * VW + DK]
                        src = tr_ps[:, h * DK:(h + 1) * DK]
                        if (kj + h) % 2 == 0:
                            nc.scalar.activation(dst, src, Copy)
                        else:
                            nc.vector.tensor_copy(dst, src)
            ones_view = vaug_sb[:].rearrange(
                "p (h k c) -> p h k c", h=HPC, k=NKT)[:, :, :, DK]
            nc.vector.memset(ones_view, 1.0)

            # ---- phase B: attention -----------------------------------
            with (
                tc.tile_pool(name="pe", bufs=2, space="PSUM") as pe_pool,
                tc.tile_pool(name="pctx", bufs=2, space="PSUM") as pctx_pool,
            ):
                for qc in range(NQ):
                    ctx_ps = []
                    for _h in range(HPC):
                        ctx_ps_h = pctx_pool.tile([VW, QC], F32, tag="ctx")
                        ctx_ps.append(ctx_ps_h)
                    nkj = 4 * qc + 4
                    for kj in range(nkj):
                        qstart = max(qc * QC, kj * P)
                        w = (qc + 1) * QC - qstart
                        e_ps = pe_pool.tile([P, HPC, QC], F32, tag="eps")
                        for h in range(HPC):
                            nc.tensor.matmul(
                                e_ps[:, h, 0:w],
                                kt_sb[h * DK:(h + 1) * DK,
                                      kj * P:(kj + 1) * P],
                                qt_sb[h * DK:(h + 1) * DK,
                                      qstart:qstart + w],
                                start=True, stop=True,
                            )
                        exp_t = work.tile([P, 2 * w], F16, tag="expt")
                        e3 = exp_t[:].rearrange("p (h f) -> p h f", h=HPC)
                        nc.scalar.activation(e3, e_ps[:, :, 0:w], Exp,
                                             bias=shift_sb[:])
                        p_t = work.tile([P, 2 * w], F16, tag="pt")
                        gwin = g_sb[:].rearrange("p (h j) -> p h j", h=HPC)[
                            :, :, 384 + qstart - kj * P:
                            384 + qstart - kj * P + w]
                        p3 = p_t[:].rearrange("p (h f) -> p h f", h=HPC)
                        nc.vector.tensor_mul(p3, e3, gwin)
                        for h in range(HPC):
                            nc.tensor.matmul(
                                ctx_ps[h][:, qstart - qc * QC:
                                          qstart - qc * QC + w],
                                vaug_sb[:, h * VW * NKT + kj * VW:
                                        h * VW * NKT + (kj + 1) * VW],
                                p_t[:, h * w:(h + 1) * w],
                                start=(kj == 0), stop=(kj == nkj - 1),
                            )
                    # normalize: ctx[dv, q] / sum[q]
                    for h in range(HPC):
                        sum_sb = norm.tile([1, QC], F32, tag="sum")
                        nc.vector.tensor_copy(sum_sb[:], ctx_ps[h][DK:DK + 1, :])
                        recip = norm.tile([1, QC], F32, tag="recip")
                        nc.vector.reciprocal_approx_fast(out=recip[:],
                                                         in_=sum_sb[:])
                        bounce = drbnc.tile([1, QC], F32, tag="bounce")
                        nc.sync.dma_start(bounce[:], recip[:])
                        bcast = norm.tile([DK, QC], F32, tag="bcast")
                        nc.sync.dma_start(
                            bcast[:], bounce[:].broadcast_to([DK, QC]))
                        nc.vector.tensor_mul(
                            ctx_sb[h * DK:(h + 1) * DK,
                                   qc * QC:(qc + 1) * QC],
                            ctx_ps[h][0:DK, :], bcast[:])

            # ---- phase C: output projection ---------------------------
            with tc.tile_pool(name="pout", bufs=4, space="PSUM") as pout:
                for do in range(D // P):
                    o_sb = outp.tile([P, S], F16, tag="osb")
                    for qc in range(NQ):
                        ps = pout.tile([P, QC], F32, tag="pout")
                        nc.tensor.matmul(
                            ps[:],
                            w_sb["wo"][:, do * P:(do + 1) * P],
                            ctx_sb[:, qc * QC:(qc + 1) * QC],
                            start=True, stop=True,
                        )
                        if (do * NQ + qc) % 2 == 0:
                            nc.scalar.activation(
                                o_sb[:, qc * QC:(qc + 1) * QC], ps[:], Copy)
                        else:
                            nc.vector.tensor_copy(
                                o_sb[:, qc * QC:(qc + 1) * QC], ps[:])
                    nc.sync.dma_start(
                        out.ap()[do * P:(do + 1) * P, :], o_sb[:])

    nc.compile()
    return nc


_NC_CACHE = None


def _get_program():
    global _NC_CACHE
    if _NC_CACHE is None:
        _NC_CACHE = _build_program()
    return _NC_CACHE


def _bucket_table():
    """T5 unidirectional bucket for distances d = q - k in [0, S).

    Mirrors the fp32 arithmetic of the reference (jnp) implementation.
    """
    import jax.numpy as jnp
    d = jnp.arange(S)
    max_exact = NUM_BUCKETS // 2
    nf = jnp.maximum(d, 1).astype(jnp.float32)
    val = max_exact + (
        jnp.log(nf / max_exact) / math.log(MAX_DISTANCE / max_exact)
        * (NUM_BUCKETS - max_exact)
    ).astype(jnp.int32)
    val = jnp.minimum(val, NUM_BUCKETS - 1)
    bucket = jnp.where(d < max_exact, d, val)
    return np.asarray(bucket)


def make_in_maps(query, key, value, Wq, bq, Wk, bk, Wv, bv, Wo, rpe_table):
    """Host-side shard prep: returns list of 8 per-core input dicts."""
    q2 = np.asarray(query, np.float32).reshape(S, D)
    k2 = np.asarray(key, np.float32).reshape(S, D)
    v2 = np.asarray(value, np.float32).reshape(S, D)
    xq_t = np.ascontiguousarray(q2.T).astype(np.float16)
    xk_t = np.ascontiguousarray(k2.T).astype(np.float16)
    xv_t = np.ascontiguousarray(v2.T).astype(np.float16)

    Wq = np.asarray(Wq, np.float32)
    Wk = np.asarray(Wk, np.float32)
    Wv = np.asarray(Wv, np.float32)
    Wo = np.asarray(Wo, np.float32)
    bq = np.asarray(bq, np.float32)
    bk = np.asarray(bk, np.float32)
    bv = np.asarray(bv, np.float32)
    rpe = np.asarray(rpe_table, np.float32)

    bucket = _bucket_table()                      # [S] int
    bias_d = rpe[bucket, :]                       # [S, H] bias by distance

    # G[p, j] = exp(bias(d)) for d = j - 384 - p, 0 where d < 0 (masked)
    dmat = np.arange(GW)[None, :] - 384 - np.arange(P)[:, None]
    valid = dmat >= 0
    dcl = np.clip(dmat, 0, S - 1)

    def chunk(w, c):          # [D, 128] slice -> [128, D] d-in-chunked layout
        sl = w[:, c * P:(c + 1) * P]
        return np.ascontiguousarray(
            sl.reshape(D // P, P, P).transpose(1, 0, 2).reshape(P, D))

    in_maps = []
    for c in range(NCORES):
        g = np.empty((P, HPC * GW), np.float16)
        for h in range(HPC):
            fexp = np.exp(bias_d[:, HPC * c + h])
            g[:, h * GW:(h + 1) * GW] = np.where(
                valid, fexp[dcl], 0.0).astype(np.float16)
        in_maps.append({
            "xq": xq_t, "xk": xk_t, "xv": xv_t,
            "wq": chunk(Wq, c).astype(np.float16),
            "wk": chunk(Wk, c).astype(np.float16),
            "wv": chunk(Wv, c).astype(np.float16),
            "wo": np.ascontiguousarray(
                Wo[c * P:(c + 1) * P, :]).astype(np.float16),
            "bq": (0.125 * bq[c * P:(c + 1) * P]).reshape(P, 1),
            "bk": bk[c * P:(c + 1) * P].reshape(P, 1).copy(),
            "bv": bv[c * P:(c + 1) * P].reshape(P, 1).copy(),
            "gt": g,
        })
    return in_maps


def assemble_output(results, bo):
    """Sum the 8 partial out^T tensors, transpose, add bo."""
    acc = np.zeros((D, S), np.float32)
    for r in results:
        acc += np.asarray(r["out"], np.float32)
    out = acc.T + np.asarray(bo, np.float32)[None, :]
    return out.reshape(1, S, D).astype(np.float32)


def kernel(query, key, value, mask, Wq, bq, Wk, bk, Wv, bv, Wo, bo,
           rpe_table):
    nc = _get_program()
    in_maps = make_in_maps(query, key, value, Wq, bq, Wk, bk, Wv, bv, Wo,
                           rpe_table)
    res = run_bass_kernel_spmd(nc, in_maps, list(range(NCORES)))
    return assemble_output(res.results, bo)
